# revision 50
# baseline (speedup 1.0000x reference)
# Trainium2 Bass kernel for AttentionPooling (segment softmax-pool).
#
# Math: reference's per-slot max subtraction cancels in the softmax, so
#   w[t,k] = exp(s_t) / D_k,  D_k = sum_{t in slot_k} exp(s_t)
#   out[k,:] = sum_{t in slot_k} exp(s_t) * proj[t,:] / D_k
# (b2 shifts every score equally so it cancels too and is dropped.)
# With A[t,k] = in_slot(t,k) * exp(s_t), both numerator and D come from one
# accumulated PE matmul per 128-row chunk:  [num | D] += A^T @ [proj | 1].
#
# The score MLP needs proj with H on partitions; rather than transposing on
# the PE (costly: PE transpose + PSUM->SBUF copy per chunk), the host ships
# proj twice: t-major bf16 for the segment matmul, h-major fp8-e3m4 for the
# score MLP (score errors only perturb softmax weights, and e3m4 with W1
# pre-scaled x16 out of its subnormal range matches bf16 accuracy: both give
# 2.0e-3 global rel err). The fp8 copy halves the score-path HBM bytes.
#
# Software pipelining: per job, the h-major slab for job j+1 is loaded before
# the t-major data of job j, so exp-weights are ready when segment data lands
# and the post-DMA tail is only the last segment chain. The final job tapers
# (4/2/1/1 chunks) to keep that tail short.
#
# Sharding: data-parallel over B; core i handles batches 2i, 2i+1.

import numpy as np
import ml_dtypes

import concourse.bacc as bacc
import concourse.tile as tile
import concourse.mybir as mybir
import concourse.bass as bass
from concourse.bass_utils import run_bass_kernel_spmd

B, T, H, K = 16, 8192, 256, 128
HQ = 64
NCORES = 8
BPC = B // NCORES          # batches per core
CH = 128                   # rows per chunk
NCH = T // CH              # 64 chunks per batch
GRP = 8                    # chunks per DMA job
SUB = 4                    # chunks per W1-matmul / exp subgroup

F32 = mybir.dt.float32
BF16 = mybir.dt.bfloat16
FP8 = mybir.dt.float8e3
W1_PRESCALE = 16.0


def make_jobs():
    jobs = []
    for b in range(BPC):
        for G in range(NCH // GRP):
            jobs.append((b, G * GRP, GRP))
    # taper the global tail so the last segment chain is short
    b, c0, n = jobs.pop()
    jobs += [(b, c0, 4), (b, c0 + 4, 2), (b, c0 + 6, 1), (b, c0 + 7, 1)]
    return jobs


def build_program():
    nc = bacc.Bacc(None, target_bir_lowering=False, debug=False)

    # t-major bf16 proj, host-tiled [b, G, p, g, h] so each partition reads
    # one contiguous 4KB run per job (DMA descriptor efficiency)
    NG = NCH // GRP
    proj = nc.dram_tensor("proj", [BPC, NG, CH, GRP, H], BF16, kind="ExternalInput")
    # h-major fp8 proj (rhs of score matmul): [b, half, h_in_half, t]
    projt = nc.dram_tensor("projt", [BPC, 2, CH, T], FP8, kind="ExternalInput")
    # starts and ends packed: [2, b, k]; int16 copy for DVE (2-byte operands
    # enable its 2x perf mode), f32 copy for GPSIMD (no int16 compare support)
    bounds = nc.dram_tensor("bounds", [2, BPC, K], mybir.dt.int16, kind="ExternalInput")
    bounds32 = nc.dram_tensor("bounds32", [2, BPC, K], F32, kind="ExternalInput")
    # W1 halves (pre-scaled x16): [half, p, hq]; W2 separate in bf16
    wpack = nc.dram_tensor("wpack", [2, CH, HQ], FP8, kind="ExternalInput")
    w2d = nc.dram_tensor("w2d", [HQ], BF16, kind="ExternalInput")
    b1 = nc.dram_tensor("b1", [HQ], F32, kind="ExternalInput")
    out = nc.dram_tensor("out", [BPC, K, H], F32, kind="ExternalOutput")

    with tile.TileContext(nc) as tc:
        with (
            tc.tile_pool(name="const", bufs=1) as const,
            tc.tile_pool(name="projg", bufs=11) as projp,
            tc.tile_pool(name="projtg", bufs=10) as ptp,
            tc.tile_pool(name="htanh", bufs=6) as htp,
            tc.tile_pool(name="amask", bufs=16) as apool,
            tc.tile_pool(name="a2steady", bufs=80) as a2p,
            tc.tile_pool(name="aprefetch", bufs=26) as prea,
            tc.tile_pool(name="gprefetch", bufs=6) as gpre,
            tc.tile_pool(name="eall", bufs=2) as epool,
            tc.tile_pool(name="outs", bufs=2) as outp,
            tc.tile_pool(name="misc", bufs=2) as miscp,
            tc.tile_pool(name="psH", bufs=2, space="PSUM") as psH,
            tc.tile_pool(name="psS", bufs=2, space="PSUM") as psS,
            tc.tile_pool(name="psSeg", bufs=2, space="PSUM") as psSeg,
        ):
            jobs = make_jobs()
            last_issued = {}
            for b_, c0_, n_ in jobs:
                last_issued[b_] = c0_ + n_ - 1
            e_alls = [
                epool.tile([CH, NCH], F32, tag="eall", name=f"e_all{b}")
                for b in range(BPC)
            ]
            segs = [
                psSeg.tile([K, H + 2], F32, tag="seg", name=f"seg{b}")
                for b in range(BPC)
            ]

            def dma_pt(b, c0, n):
                pt_tile = ptp.tile([CH, 2, GRP * CH], FP8, tag="pt")
                nc.sync.dma_start(
                    out=pt_tile[:, :, 0 : n * CH],
                    in_=bass.AP(
                        projt,
                        b * 2 * CH * T + c0 * CH,
                        [[T, CH], [CH * T, 2], [1, n * CH]],
                    ),
                )
                return pt_tile

            # kick off the first score slab before the constant loads
            pt_tiles = {0: dma_pt(*jobs[0])}

            # warm the Act function table (Tanh/Exp set) while the first
            # score slab is still in flight — the 1283ns load would otherwise
            # sit on the critical path of job 0
            warm = miscp.tile([1, 1], F32, name="warm")
            nc.gpsimd.memset(warm[:], 0.0)
            nc.scalar.activation(
                out=warm[:], in_=warm[:], func=mybir.ActivationFunctionType.Exp
            )

            # ---- constants ----
            # tcol[p, c] = p + 128*c  (t coordinate of row p in chunk c).
            # Scalar operands must be f32 for compare ops; only the tensor
            # operands (bnd int16, a tiles bf16) need 2-byte for DVE 2x mode.
            tcol = const.tile([CH, NCH], F32)
            nc.gpsimd.iota(
                tcol[:],
                pattern=[[CH, NCH]],
                base=0,
                channel_multiplier=1,
                allow_small_or_imprecise_dtypes=True,
            )

            # const loads on the SP/HWDGE queue (tiny transfers): issuing them
            # from gpsimd would delay the Pool a1 stream by ~4us of SWDGE
            # descriptor generation, and Pool is end-game critical
            wp = const.tile([CH, 2, HQ], FP8)
            nc.sync.dma_start(
                out=wp[:],
                in_=bass.AP(wpack, 0, [[HQ, CH], [CH * HQ, 2], [1, HQ]]),
            )
            w2t = const.tile([HQ, 1], BF16)
            nc.sync.dma_start(out=w2t[:], in_=bass.AP(w2d, 0, [[1, HQ], [1, 1]]))
            w2_sb = w2t[:]
            b1_sb = const.tile([HQ, 1], F32)
            nc.sync.dma_start(out=b1_sb[:], in_=bass.AP(b1, 0, [[1, HQ], [1, 1]]))

            # boundaries broadcast down all 128 partitions: [p, se, b, k]
            bnd = const.tile([CH, 2, BPC, K], mybir.dt.int16)
            nc.sync.dma_start(
                out=bnd[:],
                in_=bass.AP(bounds, 0, [[0, CH], [BPC * K, 2], [K, BPC], [1, K]]),
            )
            bnd32 = const.tile([CH, 2, BPC, K], F32)
            nc.sync.dma_start(
                out=bnd32[:],
                in_=bass.AP(bounds32, 0, [[0, CH], [BPC * K, 2], [K, BPC], [1, K]]),
            )

            def scores(b, c0, n, pt_tile):
                e_all = e_alls[b]
                hps = psH.tile([HQ, GRP, CH], F32, tag="hps")
                # matmul free dim caps at 512 f32 (one PSUM bank per accum
                # group), so fill the job's hps in SUB-chunk slabs
                for s0 in range(0, n, SUB):
                    ns = min(SUB, n - s0)
                    for half in range(2):
                        nc.tensor.matmul(
                            hps[:, s0 : s0 + ns, :],
                            wp[:, half, :],
                            pt_tile[:, half, s0 * CH : (s0 + ns) * CH],
                            start=(half == 0),
                            stop=(half == 1),
                        )
                hts = htp.tile([HQ, GRP, CH], BF16, tag="hts")
                nc.scalar.activation(
                    out=hts[:, 0:n, :],
                    in_=hps[:, 0:n, :],
                    func=mybir.ActivationFunctionType.Tanh,
                    bias=b1_sb[:],
                    scale=1.0 / W1_PRESCALE,
                )
                s_ps = psS.tile([CH, GRP], F32, tag="sps")
                for j in range(n):
                    nc.tensor.matmul(
                        s_ps[:, j : j + 1],
                        hts[:, j, :],
                        w2_sb,
                        start=True,
                        stop=True,
                    )
                nc.scalar.activation(
                    out=e_all[:, c0 : c0 + n],
                    in_=s_ps[:, 0:n],
                    func=mybir.ActivationFunctionType.Exp,
                )

            # a-gen ops alternate DVE/GPSIMD by measured cost ratio
            agen_ctr = [0]

            def agen(b, c0, n, a2pool=None):
                e_all = e_alls[b]
                a2s = []
                for g in range(n):
                    c = c0 + g
                    a1 = apool.tile([CH, K], BF16, tag="a1")
                    a2 = (a2pool or a2p).tile([CH, K], BF16, tag="a2")
                    # a2 (scalar_tensor_tensor) has no DVE fast mode (194ns)
                    # and is DVE-only; a1 (94ns on DVE at 4x, 273ns on Pool)
                    # goes 70% to Pool to balance engine busy time
                    eng1 = nc.gpsimd if (agen_ctr[0] % 10) < 7 else nc.vector
                    agen_ctr[0] += 1
                    eng2 = nc.vector
                    b1_ = bnd if eng1 is nc.vector else bnd32
                    b2_ = bnd
                    # a1[t,k] = (start_k <= t) * E_t
                    eng1.tensor_scalar(
                        out=a1[:],
                        in0=b1_[:, 0, b, :],
                        scalar1=tcol[:, c : c + 1],
                        scalar2=e_all[:, c : c + 1],
                        op0=mybir.AluOpType.is_le,
                        op1=mybir.AluOpType.mult,
                    )
                    # a2[t,k] = (end_k > t) * a1
                    eng2.scalar_tensor_tensor(
                        out=a2[:],
                        in0=b2_[:, 1, b, :],
                        scalar=tcol[:, c : c + 1],
                        in1=a1[:],
                        op0=mybir.AluOpType.is_gt,
                        op1=mybir.AluOpType.mult,
                    )
                    a2s.append(a2)
                return a2s

            def dma_g(b, c0, n, pool):
                g_tile = pool.tile([CH, GRP, H + 2], BF16, tag="g")
                G, g0 = c0 // GRP, c0 % GRP
                nc.sync.dma_start(
                    out=g_tile[:, 0:n, 0:H],
                    in_=bass.AP(
                        proj,
                        (b * (NCH // GRP) + G) * CH * GRP * H + g0 * H,
                        [[GRP * H, CH], [H, n], [1, H]],
                    ),
                )
                # ones columns via DVE memset (cheap there); on Pool it would
                # queue behind the a1 backlog and gate the seg matmuls
                nc.vector.memset(g_tile[:, 0:n, H : H + 2], 1.0)
                return g_tile

            def seg_group(b, c0, n, a2s, g_tile=None):
                seg = segs[b]
                if g_tile is None:
                    g_tile = dma_g(b, c0, n, projp)
                for g in range(n):
                    c = c0 + g
                    nc.tensor.matmul(
                        seg[:],
                        a2s[g][:],
                        g_tile[:, g, :],
                        start=(c == 0),
                        stop=(c == last_issued[b]),
                    )

            def epilogue(b):
                seg = segs[b]
                rec = miscp.tile([K, 1], F32)
                nc.vector.tensor_scalar(
                    out=rec[:],
                    in0=seg[:, H : H + 1],
                    scalar1=1e-30,
                    scalar2=None,
                    op0=mybir.AluOpType.add,
                )
                nc.vector.reciprocal(rec[:], rec[:])
                ot = outp.tile([K, H], F32)
                nc.scalar.mul(out=ot[:], in_=seg[:, 0:H], mul=rec[:])
                # issue from the Act queue: on the SP queue this DMA (which
                # waits on the whole seg chain) head-of-line blocks every
                # later pt/g slab queued behind it
                nc.scalar.dma_start(
                    out=bass.AP(out, b * K * H, [[H, K], [1, H]]), in_=ot[:]
                )

            # scores are emitted in a different order than seg: job 0 first,
            # then the tapered tail jobs (so their E-weights are computed at
            # the START of the kernel and the post-DMA tail is only the last
            # seg chains), then the middle jobs one per seg iteration.
            last_jx = {}
            for jx, (b_, c0_, n_) in enumerate(jobs):
                last_jx[b_] = jx
            # front-load the scores AND a2 masks for every job whose seg
            # matmuls land near the end of a batch chain (the 4 taper jobs,
            # the last regular job of b=1, and the last job of b=0), so the
            # post-DMA tail is only the final seg matmuls + epilogue.
            ntaper = 4
            pre = list(range(len(jobs) - ntaper - 1, len(jobs))) + [last_jx[0]]
            score_seq = [0] + pre + [
                j for j in range(1, len(jobs)) if j not in pre
            ]
            a2_map = {}
            scored = set()

            def do_score(sx):
                scores(*jobs[sx], pt_tiles.pop(sx))
                scored.add(sx)
                if sx in pre:
                    a2_map[sx] = agen(*jobs[sx], a2pool=prea)

            nlead = len(pre) + 2
            for sx in score_seq[1 : 1 + nlead]:
                pt_tiles[sx] = dma_pt(*jobs[sx])
            # prefetch the tail jobs' seg slabs too: their DMAs have no input
            # dependencies, and fetching them last would put a ~1us DMA round
            # trip per taper stage on the critical path at the very end
            g_map = {jx: dma_g(*jobs[jx], gpre) for jx in pre}
            for sx in score_seq[: 1 + nlead]:
                do_score(sx)
            nxt = [1 + nlead]
            agen_next = [0]

            def pump_scores(count):
                for _ in range(count):
                    if nxt[0] >= len(score_seq):
                        return
                    sx = score_seq[nxt[0]]
                    nxt[0] += 1
                    pt_tiles[sx] = dma_pt(*jobs[sx])
                    do_score(sx)

            def pump_agen(upto):
                # run a-gen ahead of the seg consumer so late jobs' masks
                # are never on the tail critical path
                while agen_next[0] < len(jobs) and agen_next[0] <= upto:
                    j = agen_next[0]
                    if j in a2_map or j in scored and j in pre:
                        agen_next[0] += 1
                        continue
                    if j not in scored:
                        break
                    a2_map[j] = agen(*jobs[j])
                    agen_next[0] += 1

            for jx, (b, c0, n) in enumerate(jobs):
                pump_scores(2 if jx < 6 else 1)
                pump_agen(jx + 4)
                if jx not in a2_map:
                    a2_map[jx] = agen(b, c0, n)
                seg_group(b, c0, n, a2_map.pop(jx), g_map.pop(jx, None))
                if last_jx[b] == jx:
                    epilogue(b)

    nc.compile()
    return nc


_prog_cache = None
LAST_RESULTS = None


def _get_program():
    global _prog_cache
    if _prog_cache is None:
        _prog_cache = build_program()
    return _prog_cache


def kernel(**inputs):
    proj = np.asarray(inputs["projected"], dtype=np.float32)
    bnds = np.asarray(inputs["boundaries"])
    slot = np.asarray(inputs["slot_mask"])
    W1 = np.asarray(inputs["W1"], dtype=np.float32)
    b1 = np.ascontiguousarray(np.asarray(inputs["b1"], dtype=np.float32))
    W2 = np.asarray(inputs["W2"], dtype=np.float32).reshape(HQ)

    live = slot > 0
    starts = np.where(live, bnds[..., 0], 0).astype(np.int16)     # [B, K]
    ends = np.where(live, bnds[..., 1], 0).astype(np.int16)

    proj_bf = proj.astype(ml_dtypes.bfloat16)                      # [B, T, H]
    projt_q = np.ascontiguousarray(
        proj.transpose(0, 2, 1).reshape(B, 2, CH, T)
    ).astype(ml_dtypes.float8_e3m4)                                # [B, 2, 128, T]
    # [B, T, H] -> [B, G, p, g, h]: per-partition contiguous job runs
    proj_bf = np.ascontiguousarray(
        proj_bf.reshape(B, NCH // GRP, GRP, CH, H).transpose(0, 1, 3, 2, 4)
    )

    wpack = (W1 * W1_PRESCALE).reshape(2, CH, HQ).astype(ml_dtypes.float8_e3m4)
    w2_bf = W2.astype(ml_dtypes.bfloat16)

    nc = _get_program()
    in_maps = []
    for i in range(NCORES):
        lo, hi = i * BPC, (i + 1) * BPC
        in_maps.append(
            {
                "proj": proj_bf[lo:hi],
                "projt": projt_q[lo:hi],
                "bounds": np.ascontiguousarray(
                    np.stack([starts[lo:hi], ends[lo:hi]])
                ),
                "bounds32": np.ascontiguousarray(
                    np.stack([starts[lo:hi], ends[lo:hi]]).astype(np.float32)
                ),
                "wpack": wpack,
                "w2d": w2_bf,
                "b1": b1,
            }
        )

    res = run_bass_kernel_spmd(nc, in_maps, core_ids=list(range(NCORES)))
    global LAST_RESULTS
    LAST_RESULTS = res
    outs = [r["out"] for r in res.results]
    return np.concatenate(outs, axis=0).reshape(B, K, H).astype(np.float32)

